# revision 11
# baseline (speedup 1.0000x reference)
"""Single-head attention on 8 TRN2 NeuronCores, data-parallel over batch.

Per core (one batch element b):
  x_b [2048, 768] f32 -> Q = x Wq, K = x Wk, V = x Wv (head 64)
  scores^T[k, q] = K^T-slice.T @ Q^T / 8 ; E = exp(scores);
  U = [V,1]^T-weighted sums of E give out + denominator.

v4 layout/schedule:
  - Strip 0 (x tiles 0-3) loads f32 via HWDGE (sync) which boots several
    microseconds before the SWDGE path; the idle ScalarE casts it to
    bf16. Tiles 4-15 stream via SWDGE cast-DMA f32->bf16 on gpsimd.
  - Identity/duplication matrices arrive precomputed via one small DMA
    (the gpsimd-built make_identity would gate the first transposes).
  - x^T is built by per-tile PE transpose units (6 chunks -> one PSUM
    tile -> one evac) so work flows tile-by-tile as DMAs land.
  - Projections pack A=[Wq|Wk] and B=[Wv|Wq]; K^T is duplicated into
    both partition halves with one PE matmul against [I64|I64] so score
    matmuls run as concurrent 2-way row-tiled pairs.
  - exp on ScalarE, one [128, 1024] ACTIVATE per k-tile from PSUM; the
    kt loop is software-pipelined in emission order (scores(kt) ->
    exp(kt) -> PV(kt-1) -> fillers) so exp never queues behind PV or
    phase-2 work on the PE FIFO. Strips 2-3 and the s1 K-dup/V build
    run as fillers inside qh0's PE slack; qh0's output tiles inside
    qh1's.
  - PV uses lhsT = [V, ones] (M=65); psum row 64 is the softmax
    denominator. U transposes run in bf16; outputs collect in one SBUF
    buffer per q-half and leave as a single DMA each.
"""

import numpy as np

import concourse.bass as bass
import concourse.tile as tile
from concourse import bacc, mybir
from concourse.bass_utils import run_bass_kernel_spmd

B, S, D, H = 8, 2048, 768, 64
P = 128
NT = S // P  # 16 seq tiles
NCH = D // P  # 6 emb chunks
QC = 512
N_CORES = 8
F32 = mybir.dt.float32
BF16 = mybir.dt.bfloat16
EXP = mybir.ActivationFunctionType.Exp
SCALE = float(1.0 / np.sqrt(H))


def build_kernel():
    nc = bacc.Bacc("TRN2", num_devices=N_CORES)
    x_ext = nc.declare_dram_parameter("x", [S, D], F32, isOutput=False)
    wk_ext = nc.declare_dram_parameter("Wk", [D, H], F32, isOutput=False)
    wq_ext = nc.declare_dram_parameter("Wq", [D, H], F32, isOutput=False)
    wv_ext = nc.declare_dram_parameter("Wv", [D, H], F32, isOutput=False)
    idb_ext = nc.declare_dram_parameter("idb", [P, 2, P], BF16,
                                        isOutput=False)
    out_ext = nc.declare_dram_parameter("out", [S, H], F32, isOutput=True)

    with tile.TileContext(nc) as tc:
        _body(nc, tc, x_ext, wq_ext, wk_ext, wv_ext, idb_ext, out_ext)
    nc.compile()
    return nc


def _body(nc, tc, x_ext, wq_ext, wk_ext, wv_ext, idb_ext, out_ext):
    with (
        tc.tile_pool(name="singles", bufs=1) as singles,
        tc.tile_pool(name="xn", bufs=3) as xn_pool,
        tc.tile_pool(name="et", bufs=3) as et_pool,
        tc.tile_pool(name="fin", bufs=4) as fin_pool,
        tc.tile_pool(name="ph2", bufs=2, space="PSUM") as ph2,
        tc.tile_pool(name="ss", bufs=2, space="PSUM") as ss_pool,
        tc.tile_pool(name="uu", bufs=2, space="PSUM") as u_pool,
    ):
        # ---- HWDGE (sync): identities, strip-0 f32, weights
        idb = singles.tile([P, 2, P], BF16, tag="idb")
        nc.sync.dma_start(out=idb, in_=idb_ext[:, :, :])
        ident_bf = idb[:, 0, :]
        dupI = idb[:, 1, :]  # [I64|I64] in partitions 64-127

        xf_tiles = [singles.tile([P, D], F32, name=f"xf_{st}",
                                 tag=f"xf_{st}") for st in range(4)]
        for st in range(4):
            nc.sync.dma_start(
                out=xf_tiles[st], in_=x_ext[st * P:(st + 1) * P, :])

        wq_st = singles.tile([P, NCH, H], F32, tag="wst_q")
        wk_st = singles.tile([P, NCH, H], F32, tag="wst_k")
        wv_st = singles.tile([P, NCH, H], F32, tag="wst_v")
        for w_st, w_ext in ((wq_st, wq_ext), (wk_st, wk_ext), (wv_st, wv_ext)):
            nc.sync.dma_start(
                out=w_st, in_=w_ext.rearrange("(c p) h -> p c h", p=P))

        # ---- SWDGE (gpsimd): tiles 4-15 cast-DMA f32->bf16
        xn_tiles = [xn_pool.tile([P, D], BF16, name=f"xn_{st}",
                                 tag=f"xn_{st}", bufs=1)
                    for st in range(NT)]
        for st in range(4, NT):
            nc.gpsimd.dma_start(
                out=xn_tiles[st], in_=x_ext[st * P:(st + 1) * P, :])

        # warm the exp table set; cast strip 0 on the idle scalar engine
        dummy = singles.tile([P, 8], BF16, tag="dummy")
        nc.scalar.activation(dummy, idb[:, 0, 0:8], EXP, scale=SCALE)
        for st in range(4):
            nc.scalar.copy(out=xn_tiles[st], in_=xf_tiles[st])

        # weight packs A=[Wq|Wk], B=[Wv|Wq] (DVE)
        wA = singles.tile([P, NCH, P], BF16, tag="wA")
        wB = singles.tile([P, NCH, P], BF16, tag="wB")
        nc.vector.tensor_copy(wA[:, :, 0:H], wq_st)
        nc.vector.tensor_copy(wA[:, :, H:P], wk_st)
        nc.vector.tensor_copy(wB[:, :, 0:H], wv_st)
        nc.vector.tensor_copy(wB[:, :, H:P], wq_st)

        # ---- persistent SBUF state
        xt_sb = singles.tile([P, NCH, NT, P], BF16, tag="xt_sb")  # x^T
        qkt = singles.tile([P, S], BF16, tag="qkt")   # [Q^T; K^T]
        qvt = singles.tile([P, S], BF16, tag="qvt")   # [V^T; Q^T]
        ktd = singles.tile([P, S], BF16, tag="ktd")   # K^T both halves
        vp = singles.tile([P, NT, H + 1], BF16, tag="vp")  # V' = [V, 1]
        nc.vector.memset(vp[:, :, H:H + 1], 1.0)

        # ---- phase-2 units
        def emit_trans(st):
            pst = ph2.tile([P, NCH, P], BF16, tag="ph2", name="pst")
            for c in range(NCH):
                nc.tensor.transpose(
                    pst[:, c, :], xn_tiles[st][:, c * P:(c + 1) * P],
                    ident_bf)
            if st in (6, 7):
                nc.scalar.copy(out=xt_sb[:, :, st, :], in_=pst)
            else:
                nc.vector.tensor_copy(xt_sb[:, :, st, :], pst)

        def emit_projA(sc):
            sl = slice(sc * QC, (sc + 1) * QC)
            tsl = slice(sc * 4, (sc + 1) * 4)
            psA = ph2.tile([P, QC], F32, tag="ph2", name="psA")
            for c in range(NCH):
                nc.tensor.matmul(psA, wA[:, c, :], xt_sb[:, c, tsl, :],
                                 start=(c == 0), stop=(c == NCH - 1))
            nc.vector.tensor_copy(qkt[:, sl], psA)

        def emit_projB(sc):
            sl = slice(sc * QC, (sc + 1) * QC)
            tsl = slice(sc * 4, (sc + 1) * 4)
            psB = ph2.tile([P, QC], F32, tag="ph2", name="psB")
            for c in range(NCH):
                nc.tensor.matmul(psB, wB[:, c, :], xt_sb[:, c, tsl, :],
                                 start=(c == 0), stop=(c == NCH - 1))
            nc.vector.tensor_copy(qvt[:, sl], psB)

        def emit_kdup(sc):
            sl = slice(sc * QC, (sc + 1) * QC)
            psK = ph2.tile([P, QC], F32, tag="ph2", name="psK")
            nc.tensor.matmul(psK, dupI[64:P, :], qkt[64:P, sl],
                             start=True, stop=True)
            nc.vector.tensor_copy(ktd[:, sl], psK)

        def emit_vtrans(sc, half=None):
            tiles = range(sc * 4, (sc + 1) * 4) if half is None else (
                range(sc * 4, sc * 4 + 2) if half == 0 else
                range(sc * 4 + 2, (sc + 1) * 4))
            n = len(tiles)
            psv = ph2.tile([P, 4, H], BF16, tag="ph2", name="psv")
            for i, t in enumerate(tiles):
                nc.tensor.transpose(
                    psv[:, i, :], qvt[0:H, t * P:(t + 1) * P],
                    ident_bf[:H, :H])
            t0 = tiles[0]
            nc.vector.tensor_copy(vp[:, t0:t0 + n, 0:H], psv[:, 0:n, :])

        # ---- pre-loop: strip 0 fully, strip 1 except kdup/vtrans
        for st in range(4):
            emit_trans(st)
        emit_projA(0)
        emit_projB(0)
        for st in range(4, 8):
            emit_trans(st)
        emit_kdup(0)
        emit_vtrans(0)
        emit_projA(1)
        emit_projB(1)

        # ---- output tail for one 128-row q tile; batched DMA per q-half
        ut_tiles = {}
        ob_tiles = {}

        def emit_out(qt):
            ut = ut_tiles[qt // 4]
            ob = ob_tiles[qt // 8]
            pso = ph2.tile([P, H + 1], BF16, tag="ph2", name="pso")
            nc.tensor.transpose(
                pso, ut[:, (qt % 4) * P:(qt % 4 + 1) * P],
                ident_bf[:H + 1, :H + 1])
            rcp = fin_pool.tile([P, 1], F32, tag="rcp", name="rcp")
            nc.vector.reciprocal(rcp, pso[:, H:H + 1])
            nc.vector.tensor_scalar_mul(ob[:, qt % 8, :], pso[:, 0:H], rcp)
            if qt % 8 == 7:
                half = qt // 8
                nc.sync.dma_start(
                    out=out_ext[half * 1024:(half + 1) * 1024, :].rearrange(
                        "(t p) h -> p t h", p=P),
                    in_=ob)

        # filler at key (qh, k) is emitted in pipeline group k+1; sc(kt)
        # precedes fill(kt-1) on the PE FIFO, so a unit feeding sc(kt)/PV(kt)
        # must sit at key <= kt-2 / kt-1 respectively.
        filler = {
            (0, 1): [("kd", 1, 0), ("tr", 8, 0)],
            (0, 2): [("vt", 1, None), ("tr", 9, 0)],
            (0, 3): [("tr", 10, 0), ("tr", 11, 0)],
            (0, 4): [("A", 2, 0)],
            (0, 5): [("kd", 2, 0)],
            (0, 6): [("B", 2, 0)],
            (0, 7): [("vt", 2, None), ("tr", 12, 0)],
            (0, 8): [("tr", 13, 0), ("tr", 14, 0)],
            (0, 9): [("tr", 15, 0), ("A", 3, 0)],
            (0, 10): [("kd", 3, 0)],
            (0, 11): [("B", 3, 0), ("vt", 3, 0)],
            (0, 12): [("vt", 3, 1)],
            (1, 0): [("out", 0, 0)],
            (1, 1): [("out", 1, 0)],
            (1, 2): [("out", 2, 0)],
            (1, 3): [("out", 3, 0)],
            (1, 4): [("out", 4, 0)],
            (1, 5): [("out", 5, 0)],
            (1, 6): [("out", 6, 0)],
            (1, 7): [("out", 7, 0)],
        }

        def run_filler(qh, kt):
            for kind, a1, a2 in filler.get((qh, kt), []):
                if kind == "tr":
                    emit_trans(a1)
                elif kind == "A":
                    emit_projA(a1)
                elif kind == "B":
                    emit_projB(a1)
                elif kind == "kd":
                    emit_kdup(a1)
                elif kind == "vt":
                    emit_vtrans(a1, a2)
                elif kind == "out":
                    emit_out(a1)

        # ---- main attention loops: per q-half, 16 k-tiles,
        # software-pipelined emission: sc(kt), exp(kt), PV(kt-1), fill(kt-1)
        for qh in range(2):
            q0 = qh * 1024
            U = [u_pool.tile([H + 1, QC], F32, tag="pu",
                             name=f"U{qh}_{j}") for j in range(2)]
            et_tiles = {}
            for kt in range(NT):
                ksl = slice(kt * P, (kt + 1) * P)
                ss = ss_pool.tile([P, 2, QC], F32, tag="ss", name="ss")
                nc.tensor.matmul(
                    ss[:, 0, :], ktd[0:H, ksl],
                    qkt[0:H, q0:q0 + QC], start=True, stop=True)
                nc.tensor.matmul(
                    ss[:, 1, :], ktd[H:P, ksl],
                    qvt[H:P, q0 + QC:q0 + 2 * QC], start=True, stop=True)
                et = et_pool.tile([P, 2, QC], BF16, name="et")
                nc.scalar.activation(et, ss, EXP, scale=SCALE)
                et_tiles[kt] = et
                if kt > 0:
                    for j in range(2):
                        nc.tensor.matmul(
                            U[j], vp[:, kt - 1, :], et_tiles[kt - 1][:, j, :],
                            start=(kt - 1 == 0), stop=False)
                    del et_tiles[kt - 1]
                    run_filler(qh, kt - 1)
            for j in range(2):
                nc.tensor.matmul(
                    U[j], vp[:, NT - 1, :], et_tiles[NT - 1][:, j, :],
                    start=False, stop=True)
            run_filler(qh, NT - 1)

            # evacuate U (bf16) and stage this half's output buffer
            ob_tiles[qh] = fin_pool.tile([P, 8, H], F32, tag=f"ob{qh}",
                                         name=f"ob{qh}", bufs=1)
            for j in range(2):
                ut = fin_pool.tile([H + 1, QC], BF16, tag=f"ut{j}",
                                   name=f"ut{qh}_{j}")
                nc.vector.tensor_copy(ut, U[j])
                ut_tiles[qh * 2 + j] = ut

        # qh1's own output tiles (qh0's ran as qh1 fillers)
        for qt in range(8, 16):
            emit_out(qt)


_cached_nc = None


def _make_idb():
    idb = np.zeros((P, 2, P), dtype=np.float32)
    idb[:, 0, :] = np.eye(P)
    i64 = np.eye(64)
    idb[64:P, 1, 0:64] = i64
    idb[64:P, 1, 64:P] = i64
    try:
        import ml_dtypes
        return idb.astype(ml_dtypes.bfloat16)
    except ImportError:
        # 0.0/1.0 are exact in bf16: truncate the f32 bit pattern
        return (idb.view(np.uint32) >> 16).astype(np.uint16)


def kernel(**inputs):
    global _cached_nc
    x = np.ascontiguousarray(inputs["x"], dtype=np.float32)
    wk = np.ascontiguousarray(inputs["Wk"], dtype=np.float32)
    wq = np.ascontiguousarray(inputs["Wq"], dtype=np.float32)
    wv = np.ascontiguousarray(inputs["Wv"], dtype=np.float32)
    assert x.shape == (B, S, D)

    if _cached_nc is None:
        _cached_nc = build_kernel()
    nc = _cached_nc

    idb = _make_idb()
    in_maps = [{"x": x[b], "Wk": wk, "Wq": wq, "Wv": wv, "idb": idb}
               for b in range(B)]
    res = run_bass_kernel_spmd(nc, in_maps, core_ids=list(range(N_CORES)))
    return np.stack([res.results[i]["out"] for i in range(N_CORES)], axis=0)
